# revision 35
# baseline (speedup 1.0000x reference)
"""Trainium2 Bass kernel for the LRU (Linear Recurrent Unit) nn.Module.

Math
----
Reference computes, per timestep t (T=4096, H=2048, N=1024):
    Bu_t   = B_norm @ u_t                    (complex, B_norm = (B_re+iB_im)*gamma)
    h_t    = lambda * h_{t-1} + Bu_t         (diagonal complex recurrence)
    y_t    = Re(C @ h_t) + D * u_t

Device strategy (8 NeuronCores, tensor-parallel over d_hidden N):
Each core owns NSH = N/8 = 128 channels.  With lambda_n = r_n * exp(i*theta_n)
the rotating-frame substitution g_t = exp(-i*theta*t) * h_t turns the complex
recurrence into two *real* scans
    g_t = r * g_{t-1} + exp(-i*theta*t) * Bu_t
which map 1:1 onto the VectorE tensor_tensor_scan instruction.  Rotation
tables cos(theta_n*t), sin(theta_n*t) are precomputed on host in float64.

Per core (all matmul operands bf16, accumulation + scan in f32):
  mm1  (TensorE):  Bu.T = BnT.T @ x.T        -> [NSH, T] (re,im) in PSUM
  rot-in (VectorE): w = exp(-i theta t) Bu   -> SBUF
  scan (VectorE):  g = scan(r, w)            (chunked, carried via `initial`)
  rot-out (VectorE): h = exp(+i theta t) g   -> bf16 SBUF
  mm2  (TensorE):  y_part = h_re.T @ C_re.T - h_im.T @ C_im.T  -> [T, H]
Host gathers: y = sum_c y_part_c + D * u  (float64).

The emission is software-pipelined: mm2 of chunk c is emitted after mm1 of
chunk c+1 so the TensorE never waits on the VectorE scan chain.  The last
chunk's output stores ride the two HWDGE queues (idle by then) instead of
the gpsimd software-DGE queue, shortening the kernel tail.
"""

import os

import numpy as np

T, H, N = 4096, 2048, 1024
NCORES = 8
NSH = N // NCORES  # 128 channels per core
TCH = 512          # time chunk (= max matmul moving free dim = 1 PSUM bank)
NCHUNK = T // TCH  # 8
KT = H // 128      # 16 contraction tiles in mm1
HCH = 512          # h chunk in mm2
NHC = H // HCH     # 4

_CACHE = {}

# last BassKernelResults (for test harness introspection)
last_results = None


def _build_program():
    import concourse.mybir as mybir
    from concourse import bacc
    from concourse.tile import TileContext

    F32 = mybir.dt.float32
    BF16 = mybir.dt.bfloat16
    MUL = mybir.AluOpType.mult
    ADD = mybir.AluOpType.add
    SUB = mybir.AluOpType.subtract

    nc = bacc.Bacc("TRN2", target_bir_lowering=False, debug=False,
                   num_devices=NCORES)

    xT = nc.dram_tensor("xT", [128, NCHUNK * KT * TCH], BF16,
                        kind="ExternalInput").ap()
    bn_re = nc.dram_tensor("bn_re", [128, KT * NSH], BF16,
                           kind="ExternalInput").ap()
    bn_im = nc.dram_tensor("bn_im", [128, KT * NSH], BF16,
                           kind="ExternalInput").ap()
    ct_re = nc.dram_tensor("ct_re", [NSH, H], BF16, kind="ExternalInput").ap()
    ct_in = nc.dram_tensor("ct_in", [NSH, H], BF16, kind="ExternalInput").ap()
    # merged per-chunk rotation table: [128, c, (cos|sin), TCH]
    csT = nc.dram_tensor("csT", [NSH, NCHUNK * 2 * TCH], BF16,
                         kind="ExternalInput").ap()
    rvec = nc.dram_tensor("rvec", [NSH, 1], F32, kind="ExternalInput").ap()
    ypart = nc.dram_tensor("ypart", [T, H], BF16, kind="ExternalOutput").ap()

    with TileContext(nc) as tc:
        with (
            tc.tile_pool(name="persist", bufs=1) as pp,
            tc.tile_pool(name="xin", bufs=4) as xp,
            tc.tile_pool(name="rot", bufs=2) as rp,
            tc.tile_pool(name="wbuf", bufs=3) as wp,
            tc.tile_pool(name="gbuf", bufs=3) as gp,
            tc.tile_pool(name="hbuf", bufs=3) as hp,
            tc.tile_pool(name="yout", bufs=3) as yp,
            tc.tile_pool(name="csn", bufs=3) as cp,
            tc.tile_pool(name="ps1", bufs=2, space="PSUM") as ps1,
            tc.tile_pool(name="ps2", bufs=4, space="PSUM") as ps2,
        ):
            # ---- persistent loads ----
            # One dma_start's packets feed ~one DMA engine (~24GB/s), so
            # aggregate bandwidth = concurrently-outstanding DMA instructions.
            # bn rides the scalar HWDGE queue in four pieces (concurrent with
            # chunk 0's x pieces on sync); C goes to the gpsimd queue.
            rv = pp.tile([128, 1], F32, tag="rv")
            nc.scalar.dma_start(rv[:], rvec)
            bre = pp.tile([128, KT * NSH], BF16, tag="bre")
            bim = pp.tile([128, KT * NSH], BF16, tag="bim")
            # bn pieces sized so the a=0 slices land first (~3us), with the
            # rest streaming in consumption order behind them
            for lo, hi in ((0, 2), (2, 6), (6, 11), (11, 16)):
                nc.scalar.dma_start(bre[:, lo * NSH:hi * NSH],
                                    bn_re[:, lo * NSH:hi * NSH])
                nc.scalar.dma_start(bim[:, lo * NSH:hi * NSH],
                                    bn_im[:, lo * NSH:hi * NSH])
            ctr = pp.tile([128, H], BF16, tag="ctr")
            cti = pp.tile([128, H], BF16, tag="cti")
            rbc = pp.tile([128, TCH], F32, tag="rbc")
            nc.vector.tensor_copy(rbc[:], rv[:, 0:1].broadcast_to([128, TCH]))

            def emit_persist_rest():
                # C is needed only once mm2 starts (~+20us)
                nc.gpsimd.dma_start(ctr[:], ct_re)
                nc.gpsimd.dma_start(cti[:], ct_in)

            prev_gre = prev_gim = None
            prev_cs = None
            hist = []  # pending (chunk, hre, him, csl, snl) awaiting mm2

            def emit_front(c):
                """mm1 + rotations + scans for chunk c."""
                nonlocal prev_gre, prev_gim, prev_cs
                last_c = c == NCHUNK - 1
                # chunk 0 lands as 16 per-a pieces with the issue cost split
                # over the sync and gpsimd queues (first matmul ~+7us); later
                # chunks as 4 quarters on sync, 4 chunks outstanding (bufs=4)
                # so ~16 DMA engines stay busy on x in steady state
                xt = xp.tile([128, KT * TCH], BF16, tag="xt")
                x0 = c * KT * TCH
                npc = 16 if c == 0 else 4
                QW = KT * TCH // npc
                for q in range(npc):
                    eng = nc.gpsimd if (c == 0 and q >= 8) else nc.sync
                    eng.dma_start(
                        xt[:, q * QW:(q + 1) * QW],
                        xT[:, x0 + q * QW:x0 + (q + 1) * QW],
                    )
                if c == 0:
                    emit_persist_rest()
                pre = ps1.tile([128, TCH], F32, tag="pre")
                pim = ps1.tile([128, TCH], F32, tag="pim")
                for a in range(KT):
                    xsl = xt[:, a * TCH:(a + 1) * TCH]
                    nc.tensor.matmul(
                        pre[:], bre[:, a * NSH:(a + 1) * NSH], xsl,
                        start=(a == 0), stop=(a == KT - 1),
                    )
                    nc.tensor.matmul(
                        pim[:], bim[:, a * NSH:(a + 1) * NSH], xsl,
                        start=(a == 0), stop=(a == KT - 1),
                    )
                # merged cos|sin table for this chunk, one DMA
                cs_t = cp.tile([128, 2 * TCH], BF16, tag="cs")
                nc.gpsimd.dma_start(
                    cs_t[:], csT[:, c * 2 * TCH:(c + 1) * 2 * TCH])
                csl = cs_t[:, 0:TCH]
                snl = cs_t[:, TCH:2 * TCH]
                # rotate into the r-frame: w = e^{-i theta t} * Bu
                t1 = rp.tile([128, TCH], F32, tag="t1")
                t2 = rp.tile([128, TCH], F32, tag="t2")
                wre = wp.tile([128, TCH], F32, tag="wre")
                wim = wp.tile([128, TCH], F32, tag="wim")
                gre = gp.tile([128, TCH], F32, tag="gre")
                gim = gp.tile([128, TCH], F32, tag="gim")
                hre = hp.tile([128, TCH], BF16, tag="hre")
                him = hp.tile([128, TCH], BF16, tag="him")
                # the last chunk runs rot-in/scan/rot-out per half so mm2 of
                # its first timesteps can start ~5us earlier (shorter drain)
                halves = (
                    [slice(0, TCH)] if not last_c
                    else [slice(0, TCH // 2), slice(TCH // 2, TCH)]
                )
                for hi, hs in enumerate(halves):
                    cs_h, sn_h = csl[:, hs], snl[:, hs]
                    pre_h, pim_h = pre[:, hs], pim[:, hs]
                    t1_h, t2_h = t1[:, hs], t2[:, hs]
                    nc.vector.tensor_tensor(t1_h, cs_h, pre_h, op=MUL)
                    nc.vector.tensor_tensor(t2_h, sn_h, pim_h, op=MUL)
                    nc.vector.tensor_tensor(wre[:, hs], t1_h, t2_h, op=ADD)
                    nc.vector.tensor_tensor(t1_h, cs_h, pim_h, op=MUL)
                    nc.vector.tensor_tensor(t2_h, sn_h, pre_h, op=MUL)
                    nc.vector.tensor_tensor(wim[:, hs], t1_h, t2_h, op=SUB)
                    # the two real scans
                    if hi == 0:
                        init_re = 0.0 if c == 0 else prev_gre[:, TCH - 1:TCH]
                        init_im = 0.0 if c == 0 else prev_gim[:, TCH - 1:TCH]
                    else:
                        init_re = gre[:, hs.start - 1:hs.start]
                        init_im = gim[:, hs.start - 1:hs.start]
                    nc.vector.tensor_tensor_scan(
                        gre[:, hs], rbc[:, hs], wre[:, hs], init_re, MUL, ADD)
                    nc.vector.tensor_tensor_scan(
                        gim[:, hs], rbc[:, hs], wim[:, hs], init_im, MUL, ADD)
                    # rotate back: h = e^{+i theta t} * g
                    gre_h, gim_h = gre[:, hs], gim[:, hs]
                    nc.vector.tensor_tensor(t1_h, cs_h, gre_h, op=MUL)
                    nc.vector.tensor_tensor(t2_h, sn_h, gim_h, op=MUL)
                    nc.vector.tensor_tensor(hre[:, hs], t1_h, t2_h, op=SUB)
                    nc.vector.tensor_tensor(t1_h, cs_h, gim_h, op=MUL)
                    nc.vector.tensor_tensor(t2_h, sn_h, gre_h, op=MUL)
                    nc.vector.tensor_tensor(him[:, hs], t1_h, t2_h, op=ADD)
                prev_gre, prev_gim = gre, gim
                hist.append((c, hre, him))

            def emit_back():
                """mm2 + output for the oldest pending chunk."""
                c, hre, him = hist.pop(0)
                t0 = c * TCH
                last_c = c == NCHUNK - 1
                for tt in range(TCH // 128):
                    lre = hre[:, tt * 128:(tt + 1) * 128]
                    lim = him[:, tt * 128:(tt + 1) * 128]
                    yo = yp.tile([128, H], BF16, tag="yo")
                    pos = []
                    for _ in range(NHC):
                        po = ps2.tile([128, HCH], F32, tag="po")
                        pos.append(po)
                    for hc in range(NHC):
                        nc.tensor.matmul(
                            pos[hc][:], lre, ctr[:, hc * HCH:(hc + 1) * HCH],
                            start=True, stop=False,
                        )
                    for hc in range(NHC):
                        nc.tensor.matmul(
                            pos[hc][:], lim, cti[:, hc * HCH:(hc + 1) * HCH],
                            start=False, stop=True,
                        )
                    for hc in range(NHC):
                        nc.scalar.copy(yo[:, hc * HCH:(hc + 1) * HCH],
                                       pos[hc][:])
                    # stores keep full 4KB HBM rows but are partition-split
                    # into halves on two queues, so 4 DMA engines run them
                    # concurrently; the last chunk uses the idle HWDGE
                    # queues for the fastest flush.
                    r0 = t0 + tt * 128
                    if last_c:
                        e0, e1 = nc.sync, nc.scalar
                    else:
                        e0 = nc.gpsimd
                        e1 = nc.scalar if tt % 2 == 0 else nc.gpsimd
                    e0.dma_start(ypart[r0:r0 + 64, :], yo[0:64, :])
                    e1.dma_start(ypart[r0 + 64:r0 + 128, :], yo[64:128, :])

            for c in range(NCHUNK):
                emit_front(c)
                if c >= 1:
                    emit_back()
            emit_back()

    nc.compile()
    return nc


def _arrange_bn(bn_slice):
    import ml_dtypes
    # bn_slice [NSH, H] (float64) -> [128, KT*NSH] with
    # out[p, a*NSH + n] = bn_slice[n, a*128 + p]
    bnT = bn_slice.T.astype(ml_dtypes.bfloat16)  # [H, NSH]
    return np.ascontiguousarray(
        bnT.reshape(KT, 128, NSH).transpose(1, 0, 2)).reshape(128, -1)


def _host_prep(inputs, nu, theta, gamma_log, B_re, B_im, C_re, C_im, D):
    """Float64 host-side precompute; returns per-core input maps."""
    import ml_dtypes
    BF = ml_dtypes.bfloat16
    x = np.asarray(inputs, dtype=np.float32)
    th64 = np.exp(np.asarray(theta).astype(np.float64))
    r64 = np.exp(-np.exp(np.asarray(nu).astype(np.float64)))
    gamma = np.exp(np.asarray(gamma_log).astype(np.float64))
    Bn_re = np.asarray(B_re).astype(np.float64) * gamma[:, None]
    Bn_im = np.asarray(B_im).astype(np.float64) * gamma[:, None]
    t_idx = np.arange(T, dtype=np.float64)
    phase = th64[:, None] * t_idx[None, :]
    cos_all = np.cos(phase).astype(BF)  # [N, T]
    sin_all = np.sin(phase).astype(BF)
    # merged per-chunk layout: cs_all[n, c, 0|1, t] = cos|sin(th_n*(c*TCH+t))
    cs_all = np.stack(
        [cos_all.reshape(N, NCHUNK, TCH), sin_all.reshape(N, NCHUNK, TCH)],
        axis=2).reshape(N, NCHUNK * 2 * TCH)
    # pre-arrange x into the per-chunk SBUF layout:
    # xTa[p, c, a, t] = x[c*TCH + t, a*128 + p]
    xTa = np.ascontiguousarray(
        x.reshape(NCHUNK, TCH, KT, 128).transpose(3, 0, 2, 1).astype(BF)
    ).reshape(128, -1)
    C_re = np.asarray(C_re, dtype=np.float32).astype(BF)
    C_im = np.asarray(C_im, dtype=np.float32).astype(BF)

    in_maps = []
    for c in range(NCORES):
        sl = slice(c * NSH, (c + 1) * NSH)
        in_maps.append({
            "xT": xTa,
            "bn_re": _arrange_bn(Bn_re[sl]),
            "bn_im": _arrange_bn(Bn_im[sl]),
            "ct_re": np.ascontiguousarray(C_re[:, sl].T),
            "ct_in": np.ascontiguousarray(-C_im[:, sl].T),
            "csT": np.ascontiguousarray(cs_all[sl]),
            "rvec": np.ascontiguousarray(r64[sl].astype(np.float32)[:, None]),
        })
    return in_maps


def kernel(inputs, nu, theta, gamma_log, B_re, B_im, C_re, C_im, D):
    global last_results
    from concourse.bass_utils import run_bass_kernel_spmd

    if "nc" not in _CACHE:
        _CACHE["nc"] = _build_program()
    nc = _CACHE["nc"]

    in_maps = _host_prep(
        inputs, nu, theta, gamma_log, B_re, B_im, C_re, C_im, D)

    trace = os.environ.get("LRU_TRACE") == "1"
    res = run_bass_kernel_spmd(
        nc, in_maps, core_ids=list(range(NCORES)), trace=trace)
    last_results = res

    y64 = np.zeros((T, H), np.float64)
    for r in res.results:
        y64 += r["ypart"].astype(np.float64)
    y64 += (np.asarray(D).astype(np.float64)[None, :]
            * np.asarray(inputs).astype(np.float64))
    return y64.astype(np.float32)
